# revision 34
# baseline (speedup 1.0000x reference)
"""Trainium2 Bass kernel for nn_C3DNet — data-parallel over the 10 samples on 8 cores.

Math (per sample, from the reference):
  x:(52,7,24) -conv1(6,2,2)s(2,1,2)+sig-> (24,6,12) -conv2(4,1,2)s(4,1,2)+sig-> (6,6,6)
  -avgpool2-> 27 -fc4+sig-> 80 -fc5+sig-> 200 -fc6+sig-> 676
  out = h6.reshape(13,52) @ x.reshape(52,168)  -> (13,168) -> 2184

v2 layout (all TensorE matmuls, bf16 datapath, f32 PSUM):
  * ALL input DMAs ride the SP (sync) HWDGE ring: DRAM->SBUF triggers cost
    ~5ns there, and descriptors with rows <= ~728B spread across the 16 DMA
    engines (rows >1KB serialize onto ONE engine — that was the old fc6
    stall). w6 is pre-paired host-side and column-split into 4 descriptors.
  * Scalar engine issues NO DMAs: its first instruction is a dummy sigmoid
    so walrus's ACT_TABLE_LOAD (1.28us) runs at t~6.1us, off the chain.
  * conv1 is 2 accumulated matmuls (not 4): the host stages a second,
    h-shifted copy of x on partitions 53:105 so the two kh taps stack into
    K=105; the two kw taps differ only by a +1-element rhs column offset.
  * fc6 packs chunk pairs: lhsT [K=100/101, M=104] covering i=2p,2p+1 ->
    14 matmuls instead of 26 (LDWEIGHTS rows halved). psum6 is [104, 7*NS];
    the einsum becomes 4 matmuls (partitions 0:52 = even i, 52:104 = odd i)
    and the host un-interleaves output rows.

Raw-bass (Block + explicit semaphores): this walrus build only supports ONE
attached sync-wait per Matmult/DMA instruction, so standalone wait_ge
instructions are used. Each DMA group gets its own semaphore; consumers wait
for the group's FULL credit count (16 per descriptor).
"""

import sys
from contextlib import ExitStack

sys.path.insert(0, "/opt/trn_rl_repo")

import numpy as np
import ml_dtypes

_DMA_CREDITS = 16

BF16 = ml_dtypes.bfloat16

N_CORES = 8
NS = 2  # sample slots per core
# core i handles samples ASSIGN[i]; host gathers accordingly
ASSIGN = [[0, 8], [1, 9]] + [[i, i] for i in range(2, N_CORES)]

LAST_EXEC_NS = None
LAST_RESULT = None

_BUILT = {}


def _build_nc():
    import concourse.bass as bass
    import concourse.mybir as mybir

    f32 = mybir.dt.float32
    bf16 = mybir.dt.bfloat16
    Sig = mybir.ActivationFunctionType.Sigmoid

    nc = bass.Bass()

    # x rows 0:52 = sample data, row 52 = ones (carries b1 via wb row 52),
    # rows 53:105 = x with h shifted +1 (stacks the kh=1 taps into conv1's K)
    x_d = nc.declare_dram_parameter("x", [105, NS * 168], bf16, isOutput=False)
    # wb: conv1 mm1 (cols 0:24, rows 0:105 incl b1 ones-row 52) ++ conv1 mm2
    # (cols 24:48) ++ conv2 (cols 48:60, rows 0:25 incl b2 ones-row 24)
    wb_d = nc.declare_dram_parameter("wb", [105, 60], bf16, isOutput=False)
    # w4p row 6 = b4 in the j=0 block, zeros elsewhere
    w4p_d = nc.declare_dram_parameter("w4p", [12, 720], bf16, isOutput=False)
    w5t_d = nc.declare_dram_parameter("w5t", [86, 200], bf16, isOutput=False)
    # w6a = k 0:100 of w6.T, w6b = k 100:200 + b6 ones-row at row 100.
    # Column halves are SEPARATE contiguous tensors: the HWDGE only
    # row-splits contiguous descriptors with rows <= ~1KB across its 16
    # engines; a strided column-slice lands whole on one engine.
    w6a1_d = nc.declare_dram_parameter("w6a1", [106, 338], bf16, isOutput=False)
    w6a2_d = nc.declare_dram_parameter("w6a2", [106, 338], bf16, isOutput=False)
    w6b1_d = nc.declare_dram_parameter("w6b1", [106, 338], bf16, isOutput=False)
    w6b2_d = nc.declare_dram_parameter("w6b2", [106, 338], bf16, isOutput=False)
    out_d = nc.declare_dram_parameter("out", [13, NS * 168], f32, isOutput=True)

    es = ExitStack()

    def sb(name, shape, dt=bf16):
        return es.enter_context(nc.sbuf_tensor(name, shape, dt))

    def pt(name, shape):
        return es.enter_context(nc.psum_tensor(name, shape, f32))

    with es:
        x_t = sb("x_t", [105, NS * 168])
        wb_t = sb("wb_t", [105, 60])
        w4p_t = sb("w4p_t", [12, 720])
        w5t_t = sb("w5t_t", [86, 200])
        w6a_t = sb("w6a_t", [106, 676])
        w6b_t = sb("w6b_t", [106, 676])
        h1_t = sb("h1_t", [25, NS * 72])   # row 24 = ones (b2 rides wb col-block row 24)
        h2_t = sb("h2_t", [6, NS * 36])
        tmp6_t = sb("tmp6_t", [6, NS * 18])
        pool_t = sb("pool_t", [7, NS * 9])  # row 6 = ones (b4 rides w4p row 6)
        h4_t = sb("h4_t", [81, NS])         # row 80 = ones (b5 rides w5t row 80)
        t01 = sb("t01", [101, 2 * NS])      # cols 0:2 = t0, 2:4 = t1; row 100 = ones
        h6_t = sb("h6_t", [52, 13 * NS])    # cols = i*NS + s; filled by 2 strided ACTs
        out_t = sb("out_t", [13, NS * 168], f32)
        scr_t = sb("scr_t", [1, 2])         # bf16: table-preload dummy output
        zb_t = sb("zb_t", [104, 1], f32)    # zero bias for all sigmoids

        psum1 = pt("psum1", [24, NS * 72])
        psum2 = pt("psum2", [6, NS * 36])
        psum4 = pt("psum4", [80, NS])
        psum5 = pt("psum5", [100, 2 * NS])
        psum6 = pt("psum6", [52, 13 * NS])
        psume = pt("psume", [13, NS * 168])
        psum_scr = pt("psum_scr", [1, 2])

        dsA = es.enter_context(nc.semaphore("dsA"))    # x
        dsB = es.enter_context(nc.semaphore("dsB"))    # wb
        dsE = es.enter_context(nc.semaphore("dsE"))    # w4p halves
        dsF = es.enter_context(nc.semaphore("dsF"))    # w5t
        dsG = es.enter_context(nc.semaphore("dsG"))    # w6 quarters
        dsO = es.enter_context(nc.semaphore("dsO"))    # output (no waiter)
        ssem = es.enter_context(nc.semaphore("ssem"))  # Pool preamble memsets done
        ssev = es.enter_context(nc.semaphore("ssev"))  # DVE zb memset done
        psem = es.enter_context(nc.semaphore("psem"))
        asem = es.enter_context(nc.semaphore("asem"))
        vsem = es.enter_context(nc.semaphore("vsem"))

        with nc.Block() as block:
            hoist = nc._hoist_insts = []

            @block.sync
            def _(sync):
                # DRAM->SBUF triggers cost ~700ns apiece, and a queue's engine
                # assignment degrades under congestion — so spread tensors
                # over all three rings, <=3 input descriptors each, with rows
                # <=720B (bigger rows land on a single DMA engine).
                hoist.append(sync.dma_start(out=x_t[:], in_=x_d[:]).then_inc(dsA, 16))
                hoist.append(sync.dma_start(out=w4p_t[:], in_=w4p_d[:]).then_inc(dsE, 16))
                hoist.append(sync.dma_start(out=w6b_t[:, 0:338], in_=w6b1_d[:]).then_inc(dsG, 16))
                hoist.append(sync.dma_start(out=w6b_t[:, 338:676], in_=w6b2_d[:]).then_inc(dsG, 16))
                sync.wait_ge(asem, 7)
                # contiguous store ([13, NS*168] both sides); host untangles
                # the (i, s, w) -> (s, i*168+w) layout.
                sync.dma_start(out=out_d[:, :], in_=out_t[:]).then_inc(dsO, 16)

            @block.gpsimd
            def _(gpsimd):
                # wb rides SWDGE ahead of the ones-row memsets so conv1 isn't
                # gated on the sync ring draining x first; Pool is idle after
                hoist.append(gpsimd.dma_start(out=wb_t[:], in_=wb_d[:]).then_inc(dsB, 16))
                hoist.append(gpsimd.memset(h1_t[:], 1.0))
                hoist.append(gpsimd.memset(pool_t[:], 1.0))
                hoist.append(gpsimd.memset(h4_t[:], 1.0))
                hoist.append(gpsimd.memset(t01[:], 1.0).then_inc(ssem))

            @block.vector
            def _(vector):
                hoist.append(vector.memset(zb_t[:], 0.0).then_inc(ssev))
                # pooling over (h, w) as two strided adds, after sigmoid-2.
                # The DVE pipelines back-to-back instructions, so add2 needs a
                # real semaphore on add1 (RAW on tmp6) — program order is NOT
                # enough.
                vector.wait_ge(ssem, 1)
                vector.wait_ge(asem, 2)
                h24 = h2_t[:].rearrange("p (s h w) -> p s h w", s=NS, h=6, w=6)
                t64 = tmp6_t[:].rearrange("p (s h w) -> p s h w", s=NS, h=6, w=3)
                vector.tensor_add(t64[:], h24[:, :, :, 0:5:2], h24[:, :, :, 1:6:2]).then_inc(vsem)  # 1
                vector.wait_ge(vsem, 1)
                p64 = pool_t[0:6, :].rearrange("p (s h w) -> p s h w", s=NS, h=3, w=3)
                vector.tensor_add(
                    p64[:], t64[:, :, 0:5:2, :], t64[:, :, 1:6:2, :]
                ).then_inc(vsem)  # 2

            @block.scalar
            def _(scalar):
                scalar.wait_ge(ssev, 1)
                # dummy sigmoid FIRST IN THIS BASIC BLOCK: walrus tracks ACT
                # tables per-bb, so the preload must live in the same bb as
                # the real sigmoids. psum_scr is read uninitialized on
                # purpose — the output is never consumed.
                scalar.activation(scr_t[:], psum_scr[:], Sig, bias=zb_t[0:1, :])
                # w5t/w6a triggers ride the (otherwise idle) activation ring
                # after the table preload; fc5/fc6 need them at ~13us
                scalar.dma_start(out=w5t_t[:], in_=w5t_d[:]).then_inc(dsF, 16)
                scalar.dma_start(out=w6a_t[:, 0:338], in_=w6a1_d[:]).then_inc(dsG, 16)
                scalar.dma_start(out=w6a_t[:, 338:676], in_=w6a2_d[:]).then_inc(dsG, 16)
                scalar.wait_ge(ssem, 1)
                scalar.wait_ge(psem, 1)
                scalar.activation(h1_t[0:24, :], psum1[:], Sig, bias=zb_t[0:24, :]).then_inc(asem)  # 1
                scalar.wait_ge(psem, 2)
                scalar.activation(h2_t[:], psum2[:], Sig, bias=zb_t[0:6, :]).then_inc(asem)  # 2
                scalar.wait_ge(psem, 3)
                scalar.activation(h4_t[0:80, :], psum4[:], Sig, bias=zb_t[0:80, :]).then_inc(asem)  # 3
                scalar.wait_ge(psem, 5)
                scalar.activation(t01[0:100, :], psum5[:], Sig, bias=zb_t[0:100, :]).then_inc(asem)  # 4
                scalar.wait_ge(psem, 6)
                scalar.activation(h6_t[:], psum6[:], Sig, bias=zb_t[0:52, :]).then_inc(asem)  # 5
                scalar.wait_ge(psem, 7)
                scalar.copy(out_t[:, 0:168], psume[:, 0:168]).then_inc(asem)  # 6
                scalar.wait_ge(psem, 8)
                scalar.copy(out_t[:, 168:336], psume[:, 168:336]).then_inc(asem)  # 7

            @block.tensor
            def _(tensor):
                # conv1: 2 accumulated matmuls; K=105 = taps kh=0 (rows 0:53,
                # incl. b1 ones-row 52) + kh=1 (rows 53:105, h-shifted x).
                # The kw taps differ by a +1-element rhs column offset.
                tensor.wait_ge(dsA, _DMA_CREDITS)
                tensor.wait_ge(dsB, _DMA_CREDITS)
                x4 = x_t[:].rearrange("p (s h w) -> p s h w", s=NS, h=7, w=24)
                for kw in range(2):
                    mm = tensor.matmul(
                        psum1[:],
                        wb_t[:, kw * 24 : (kw + 1) * 24],
                        x4[:, :, 0:6, kw : kw + 23 : 2],
                        start=(kw == 0),
                        stop=(kw == 1),
                    )
                    if kw == 1:
                        mm.then_inc(psem)  # psem 1
                # conv2: K=25 incl. the b2 ones-row
                tensor.wait_ge(asem, 1)
                h14 = h1_t[:].rearrange("p (s h w) -> p s h w", s=NS, h=6, w=12)
                for kw in range(2):
                    mm = tensor.matmul(
                        psum2[:],
                        wb_t[0:25, 48 + kw * 6 : 48 + (kw + 1) * 6],
                        h14[:, :, :, kw : kw + 11 : 2],
                        start=(kw == 0),
                        stop=(kw == 1),
                    )
                    if kw == 1:
                        mm.then_inc(psem)  # psem 2
                # fc4: 9 (hp,wp) matmuls vs the h/w-pooled tile; d-pooling and
                # /8 live in w4p; j=0 has K=7 incl. the b4 ones-row
                tensor.wait_ge(vsem, 2)
                tensor.wait_ge(dsE, 16)
                pool4 = pool_t[:].rearrange("p (s j) -> p s j", s=NS, j=9)
                for j in range(9):
                    kk = 7 if j == 0 else 6
                    mm = tensor.matmul(
                        psum4[:],
                        w4p_t[0:kk, j * 80 : (j + 1) * 80],
                        pool4[0:kk, :, j],
                        start=(j == 0),
                        stop=(j == 8),
                    )
                    if j == 8:
                        mm.then_inc(psem)  # psem 3
                # fc5
                tensor.wait_ge(asem, 3)
                tensor.wait_ge(dsF, 16)
                tensor.matmul(
                    psum5[:, 0:NS], w5t_t[0:81, 0:100], h4_t[:], start=True, stop=True
                ).then_inc(psem)  # psem 4
                tensor.matmul(
                    psum5[:, NS : 2 * NS], w5t_t[0:81, 100:200], h4_t[:], start=True, stop=True
                ).then_inc(psem)  # psem 5
                # fc6: 13 i-chunks x 2 k-halves (M=52 keeps LDWEIGHTS at its
                # 2-rows/cycle fast path; M>64 halves load throughput)
                tensor.wait_ge(asem, 4)
                tensor.wait_ge(dsG, 64)
                for i in range(13):
                    tensor.matmul(
                        psum6[:, i * NS : (i + 1) * NS],
                        w6a_t[0:100, i * 52 : (i + 1) * 52],
                        t01[0:100, 0:NS],
                        start=True,
                        stop=False,
                    )
                    mm = tensor.matmul(
                        psum6[:, i * NS : (i + 1) * NS],
                        w6b_t[0:101, i * 52 : (i + 1) * 52],
                        t01[:, NS : 2 * NS],
                        start=False,
                        stop=True,
                    )
                    if i == 12:
                        mm.then_inc(psem)  # psem 6
                # einsum
                tensor.wait_ge(asem, 5)
                h6v = h6_t[:].rearrange("p (i s) -> p s i", s=NS)
                for s in range(NS):
                    tensor.matmul(
                        psume[:, s * 168 : (s + 1) * 168],
                        h6v[:, s, :],
                        x_t[0:52, s * 168 : (s + 1) * 168],
                        start=True,
                        stop=True,
                    ).then_inc(psem)  # psem 7, 8

    _strip_entry_barrier(nc)
    return nc


def _strip_entry_barrier(nc):
    f = nc.m.functions[0]
    bbs = {bb.name: bb for bb in f.blocks}
    main = bbs["main"]
    # 1) drop the init all-engine barrier (nothing reads the const-AP tiles)
    main.instructions = [
        i
        for i in main.instructions
        if not (
            i.name.startswith("barrier_")
            or getattr(i, "opcode", "") == "Drain"
            or type(i).__name__ == "InstDrain"
        )
    ]
    # 2) hoist the input-DMA triggers into main so transfers start during the
    #    preamble, before the Block-entry rendezvous
    hoisted = {bi.ins.name for bi in getattr(nc, "_hoist_insts", [])}
    if hoisted:
        moved = []
        for bb in f.blocks:
            if bb.name == "main" or not bb.instructions:
                continue
            keep = []
            for i in bb.instructions:
                (moved if i.name in hoisted else keep).append(i)
            if len(keep) != len(bb.instructions):
                bb.instructions = keep
        # insert at the very top of main (after the entry Call): the DMA
        # triggers use only immediates + the parameter table, not the
        # preamble registers
        insts = main.instructions
        main.instructions = insts[:1] + moved + insts[1:]


def _prep_weights(w1, b1, w2, b2, w4, b4, w5, b5, w6, b6):
    f = np.float32
    w1v = np.asarray(w1, f)[0, 0]  # (6,2,2)
    w2v = np.asarray(w2, f)[0, 0, :, 0, :]  # (4,2)
    w4 = np.asarray(w4, f)
    w5 = np.asarray(w5, f)
    w6 = np.asarray(w6, f)
    b1 = np.asarray(b1, f)
    b2 = np.asarray(b2, f)
    b4 = np.asarray(b4, f)
    b5 = np.asarray(b5, f)
    b6 = np.asarray(b6, f)

    wb = np.zeros((105, 60), f)
    for kw in range(2):
        for d in range(24):
            for kd in range(6):
                wb[2 * d + kd, kw * 24 + d] = w1v[kd, 0, kw]
                wb[53 + 2 * d + kd, kw * 24 + d] = w1v[kd, 1, kw]
    wb[52, 0:24] = b1[0]  # ones-row bias, kw=0 block only
    for kd in range(4):
        for kw in range(2):
            for d in range(6):
                wb[4 * d + kd, 48 + kw * 6 + d] = w2v[kd, kw]
    wb[24, 48:54] = b2[0]  # ones-row bias, kw=0 block only

    w4r = w4.reshape(80, 3, 3, 3) / 8.0
    w4q = np.transpose(w4r, (1, 2, 3, 0)).reshape(3, 720)
    w4p = np.zeros((12, 720), f)
    w4p[0:6:2, :] = w4q
    w4p[1:6:2, :] = w4q
    w4p[6, 0:80] = b4  # ones-row bias, j=0 block only

    w5t = np.zeros((86, 200), f)
    w5t[0:80, :] = w5.T
    w5t[80, :] = b5

    w6a = np.zeros((106, 676), f)
    w6a[0:100, :] = w6[:, 0:100].T
    w6b = np.zeros((106, 676), f)
    w6b[0:100, :] = w6[:, 100:200].T
    w6b[100, :] = b6

    def _bf(a):
        return np.ascontiguousarray(a.astype(BF16))

    return dict(
        wb=_bf(wb),
        w4p=_bf(w4p),
        w5t=_bf(w5t),
        w6a1=_bf(w6a[:, 0:338]),
        w6a2=_bf(w6a[:, 338:676]),
        w6b1=_bf(w6b[:, 0:338]),
        w6b2=_bf(w6b[:, 338:676]),
    )


def kernel(x, w1, b1, w2, b2, w4, b4, w5, b5, w6, b6, _trace=False):
    global LAST_EXEC_NS, LAST_RESULT
    from concourse.bass_utils import run_bass_kernel_spmd

    if "nc" not in _BUILT:
        _BUILT["nc"] = _build_nc()
    nc = _BUILT["nc"]

    xs = np.ascontiguousarray(np.asarray(x, np.float32).reshape(10, 52, 7, 24))
    # h-shifted copy for the stacked kh=1 conv1 taps
    xsh = np.zeros_like(xs)
    xsh[:, :, 0:6, :] = xs[:, :, 1:7, :]
    xs2 = xs.reshape(10, 52, 168)
    xsh2 = xsh.reshape(10, 52, 168)
    wd = _prep_weights(w1, b1, w2, b2, w4, b4, w5, b5, w6, b6)

    in_maps = []
    for i in range(N_CORES):
        xc = np.ones((105, NS * 168), np.float32)
        xc[0:52] = np.transpose(
            np.stack([xs2[a] for a in ASSIGN[i]]), (1, 0, 2)
        ).reshape(52, NS * 168)
        xc[53:105] = np.transpose(
            np.stack([xsh2[a] for a in ASSIGN[i]]), (1, 0, 2)
        ).reshape(52, NS * 168)
        xc = np.ascontiguousarray(xc.astype(BF16))
        m = {"x": xc}
        m.update(wd)
        in_maps.append(m)

    res = run_bass_kernel_spmd(nc, in_maps, core_ids=list(range(N_CORES)), trace=_trace)
    LAST_EXEC_NS = res.exec_time_ns
    LAST_RESULT = res

    out = np.zeros((10, 2184), np.float32)
    for i in range(N_CORES):
        o = res.results[i]["out"].reshape(13, NS, 168)
        for slot, b in enumerate(ASSIGN[i]):
            out[b] = o[:, slot, :].reshape(2184)
    return out


# revision 37
# speedup vs baseline: 1.2854x; 1.2854x over previous
"""Trainium2 Bass kernel for nn_C3DNet — data-parallel over the 10 samples on 8 cores.

Math (per sample, from the reference):
  x:(52,7,24) -conv1(6,2,2)s(2,1,2)+sig-> (24,6,12) -conv2(4,1,2)s(4,1,2)+sig-> (6,6,6)
  -avgpool2-> 27 -fc4+sig-> 80 -fc5+sig-> 200 -fc6+sig-> 676
  out = h6.reshape(13,52) @ x.reshape(52,168)  -> (13,168) -> 2184

v2 layout (all TensorE matmuls, bf16 datapath, f32 PSUM):
  * ALL input DMAs ride the SP (sync) HWDGE ring: DRAM->SBUF triggers cost
    ~5ns there, and descriptors with rows <= ~728B spread across the 16 DMA
    engines (rows >1KB serialize onto ONE engine — that was the old fc6
    stall). w6 is pre-paired host-side and column-split into 4 descriptors.
  * Scalar engine issues NO DMAs: its first instruction is a dummy sigmoid
    so walrus's ACT_TABLE_LOAD (1.28us) runs at t~6.1us, off the chain.
  * conv1 is 2 accumulated matmuls (not 4): the host stages a second,
    h-shifted copy of x on partitions 53:105 so the two kh taps stack into
    K=105; the two kw taps differ only by a +1-element rhs column offset.
  * fc6 packs chunk pairs: lhsT [K=100/101, M=104] covering i=2p,2p+1 ->
    14 matmuls instead of 26 (LDWEIGHTS rows halved). psum6 is [104, 7*NS];
    the einsum becomes 4 matmuls (partitions 0:52 = even i, 52:104 = odd i)
    and the host un-interleaves output rows.

Raw-bass (Block + explicit semaphores): this walrus build only supports ONE
attached sync-wait per Matmult/DMA instruction, so standalone wait_ge
instructions are used. Each DMA group gets its own semaphore; consumers wait
for the group's FULL credit count (16 per descriptor).
"""

import sys
from contextlib import ExitStack

sys.path.insert(0, "/opt/trn_rl_repo")

import numpy as np
import ml_dtypes

_DMA_CREDITS = 16

BF16 = ml_dtypes.bfloat16

N_CORES = 8
NS = 2  # sample slots per core
# core i handles samples ASSIGN[i]; host gathers accordingly
ASSIGN = [[0, 8], [1, 9]] + [[i, i] for i in range(2, N_CORES)]

LAST_EXEC_NS = None
LAST_RESULT = None

_BUILT = {}


def _build_nc():
    import concourse.bass as bass
    import concourse.mybir as mybir

    f32 = mybir.dt.float32
    bf16 = mybir.dt.bfloat16
    Sig = mybir.ActivationFunctionType.Sigmoid

    nc = bass.Bass()

    # x rows 0:52 = sample data, row 52 = ones (carries b1 via wb row 52),
    # rows 53:105 = x with h shifted +1 (stacks the kh=1 taps into conv1's K)
    x_d = nc.declare_dram_parameter("x", [105, NS * 168], bf16, isOutput=False)
    # wb: conv1 mm1 (cols 0:24, rows 0:105 incl b1 ones-row 52) ++ conv1 mm2
    # (cols 24:48) ++ conv2 (cols 48:60, rows 0:25 incl b2 ones-row 24)
    wb_d = nc.declare_dram_parameter("wb", [105, 60], bf16, isOutput=False)
    # w4p row 6 = b4 in the j=0 block, zeros elsewhere
    w4p_d = nc.declare_dram_parameter("w4p", [12, 720], bf16, isOutput=False)
    # Row counts are padded so the HWDGE row-splits each descriptor evenly
    # across its 16 engines: rows must divide by ceil(rows/16) (96 = 16x6,
    # 112 = 16x7; 86 or 106 land whole on ONE engine at ~23GB/s).
    # Column halves are SEPARATE contiguous tensors (strided column-slice
    # descriptors also serialize onto one engine).
    w5t_d = nc.declare_dram_parameter("w5t", [96, 200], bf16, isOutput=False)
    # w6a = k 0:100 of w6.T, w6b = k 100:200 + b6 ones-row at row 100
    w6a1_d = nc.declare_dram_parameter("w6a1", [112, 338], bf16, isOutput=False)
    w6a2_d = nc.declare_dram_parameter("w6a2", [112, 338], bf16, isOutput=False)
    w6b1_d = nc.declare_dram_parameter("w6b1", [112, 338], bf16, isOutput=False)
    w6b2_d = nc.declare_dram_parameter("w6b2", [112, 338], bf16, isOutput=False)
    out_d = nc.declare_dram_parameter("out", [13, NS * 168], f32, isOutput=True)

    es = ExitStack()

    def sb(name, shape, dt=bf16):
        return es.enter_context(nc.sbuf_tensor(name, shape, dt))

    def pt(name, shape):
        return es.enter_context(nc.psum_tensor(name, shape, f32))

    with es:
        x_t = sb("x_t", [105, NS * 168])
        wb_t = sb("wb_t", [105, 60])
        w4p_t = sb("w4p_t", [12, 720])
        w5t_t = sb("w5t_t", [96, 200])
        w6a_t = sb("w6a_t", [112, 676])
        w6b_t = sb("w6b_t", [112, 676])
        h1_t = sb("h1_t", [25, NS * 72])   # row 24 = ones (b2 rides wb col-block row 24)
        h2_t = sb("h2_t", [6, NS * 36])
        tmp6_t = sb("tmp6_t", [6, NS * 18])
        pool_t = sb("pool_t", [7, NS * 9])  # row 6 = ones (b4 rides w4p row 6)
        h4_t = sb("h4_t", [81, NS])         # row 80 = ones (b5 rides w5t row 80)
        t01 = sb("t01", [101, 2 * NS])      # cols 0:2 = t0, 2:4 = t1; row 100 = ones
        h6_t = sb("h6_t", [52, 13 * NS])    # cols = i*NS + s; filled by 2 strided ACTs
        out_t = sb("out_t", [13, NS * 168], f32)
        scr_t = sb("scr_t", [1, 2])         # bf16: table-preload dummy output
        zb_t = sb("zb_t", [104, 1], f32)    # zero bias for all sigmoids

        psum1 = pt("psum1", [24, NS * 72])
        psum2 = pt("psum2", [6, NS * 36])
        psum4 = pt("psum4", [80, NS])
        psum5 = pt("psum5", [100, 2 * NS])
        psum6 = pt("psum6", [52, 13 * NS])
        psume = pt("psume", [13, NS * 168])
        psum_scr = pt("psum_scr", [1, 2])

        dsA = es.enter_context(nc.semaphore("dsA"))    # x
        dsB = es.enter_context(nc.semaphore("dsB"))    # wb
        dsE = es.enter_context(nc.semaphore("dsE"))    # w4p halves
        dsF = es.enter_context(nc.semaphore("dsF"))    # w5t
        dsG = es.enter_context(nc.semaphore("dsG"))    # w6 quarters
        dsO = es.enter_context(nc.semaphore("dsO"))    # output (no waiter)
        ssem = es.enter_context(nc.semaphore("ssem"))  # Pool preamble memsets done
        ssev = es.enter_context(nc.semaphore("ssev"))  # DVE zb memset done
        psem = es.enter_context(nc.semaphore("psem"))
        asem = es.enter_context(nc.semaphore("asem"))
        vsem = es.enter_context(nc.semaphore("vsem"))

        with nc.Block() as block:
            hoist = nc._hoist_insts = []

            @block.sync
            def _(sync):
                # DRAM->SBUF triggers cost ~700ns apiece, and a queue's engine
                # assignment degrades under congestion — so spread tensors
                # over all three rings, <=3 input descriptors each, with rows
                # <=720B (bigger rows land on a single DMA engine).
                hoist.append(sync.dma_start(out=x_t[:], in_=x_d[:]).then_inc(dsA, 16))
                hoist.append(sync.dma_start(out=w4p_t[:], in_=w4p_d[:]).then_inc(dsE, 16))
                hoist.append(sync.dma_start(out=w6b_t[:, 0:338], in_=w6b1_d[:]).then_inc(dsG, 16))
                hoist.append(sync.dma_start(out=w6b_t[:, 338:676], in_=w6b2_d[:]).then_inc(dsG, 16))
                sync.wait_ge(asem, 7)
                # contiguous store ([13, NS*168] both sides); host untangles
                # the (i, s, w) -> (s, i*168+w) layout.
                sync.dma_start(out=out_d[:, :], in_=out_t[:]).then_inc(dsO, 16)

            @block.gpsimd
            def _(gpsimd):
                # wb rides SWDGE ahead of the ones-row memsets so conv1 isn't
                # gated on the sync ring draining x first; Pool is idle after
                hoist.append(gpsimd.dma_start(out=wb_t[:], in_=wb_d[:]).then_inc(dsB, 16))
                hoist.append(gpsimd.memset(h1_t[:], 1.0))
                hoist.append(gpsimd.memset(pool_t[:], 1.0))
                hoist.append(gpsimd.memset(h4_t[:], 1.0))
                hoist.append(gpsimd.memset(t01[:], 1.0).then_inc(ssem))

            @block.vector
            def _(vector):
                hoist.append(vector.memset(zb_t[:], 0.0).then_inc(ssev))
                # pooling over (h, w) as two strided adds, after sigmoid-2.
                # The DVE pipelines back-to-back instructions, so add2 needs a
                # real semaphore on add1 (RAW on tmp6) — program order is NOT
                # enough.
                vector.wait_ge(ssem, 1)
                vector.wait_ge(asem, 2)
                h24 = h2_t[:].rearrange("p (s h w) -> p s h w", s=NS, h=6, w=6)
                t64 = tmp6_t[:].rearrange("p (s h w) -> p s h w", s=NS, h=6, w=3)
                vector.tensor_add(t64[:], h24[:, :, :, 0:5:2], h24[:, :, :, 1:6:2]).then_inc(vsem)  # 1
                vector.wait_ge(vsem, 1)
                p64 = pool_t[0:6, :].rearrange("p (s h w) -> p s h w", s=NS, h=3, w=3)
                vector.tensor_add(
                    p64[:], t64[:, :, 0:5:2, :], t64[:, :, 1:6:2, :]
                ).then_inc(vsem)  # 2

            @block.scalar
            def _(scalar):
                scalar.wait_ge(ssev, 1)
                # dummy sigmoid FIRST IN THIS BASIC BLOCK: walrus tracks ACT
                # tables per-bb, so the preload must live in the same bb as
                # the real sigmoids. psum_scr is read uninitialized on
                # purpose — the output is never consumed.
                scalar.activation(scr_t[:], psum_scr[:], Sig, bias=zb_t[0:1, :])
                # w5t/w6a triggers ride the (otherwise idle) activation ring
                # after the table preload; fc5/fc6 need them at ~13us
                scalar.dma_start(out=w5t_t[:], in_=w5t_d[:]).then_inc(dsF, 16)
                scalar.dma_start(out=w6a_t[:, 0:338], in_=w6a1_d[:]).then_inc(dsG, 16)
                scalar.dma_start(out=w6a_t[:, 338:676], in_=w6a2_d[:]).then_inc(dsG, 16)
                scalar.wait_ge(ssem, 1)
                scalar.wait_ge(psem, 1)
                scalar.activation(h1_t[0:24, :], psum1[:], Sig, bias=zb_t[0:24, :]).then_inc(asem)  # 1
                scalar.wait_ge(psem, 2)
                scalar.activation(h2_t[:], psum2[:], Sig, bias=zb_t[0:6, :]).then_inc(asem)  # 2
                scalar.wait_ge(psem, 3)
                scalar.activation(h4_t[0:80, :], psum4[:], Sig, bias=zb_t[0:80, :]).then_inc(asem)  # 3
                scalar.wait_ge(psem, 5)
                scalar.activation(t01[0:100, :], psum5[:], Sig, bias=zb_t[0:100, :]).then_inc(asem)  # 4
                scalar.wait_ge(psem, 6)
                scalar.activation(h6_t[:], psum6[:], Sig, bias=zb_t[0:52, :]).then_inc(asem)  # 5
                scalar.wait_ge(psem, 7)
                scalar.copy(out_t[:, 0:168], psume[:, 0:168]).then_inc(asem)  # 6
                scalar.wait_ge(psem, 8)
                scalar.copy(out_t[:, 168:336], psume[:, 168:336]).then_inc(asem)  # 7

            @block.tensor
            def _(tensor):
                # conv1: 2 accumulated matmuls; K=105 = taps kh=0 (rows 0:53,
                # incl. b1 ones-row 52) + kh=1 (rows 53:105, h-shifted x).
                # The kw taps differ by a +1-element rhs column offset.
                tensor.wait_ge(dsA, _DMA_CREDITS)
                tensor.wait_ge(dsB, _DMA_CREDITS)
                x4 = x_t[:].rearrange("p (s h w) -> p s h w", s=NS, h=7, w=24)
                for kw in range(2):
                    mm = tensor.matmul(
                        psum1[:],
                        wb_t[:, kw * 24 : (kw + 1) * 24],
                        x4[:, :, 0:6, kw : kw + 23 : 2],
                        start=(kw == 0),
                        stop=(kw == 1),
                    )
                    if kw == 1:
                        mm.then_inc(psem)  # psem 1
                # conv2: K=25 incl. the b2 ones-row
                tensor.wait_ge(asem, 1)
                h14 = h1_t[:].rearrange("p (s h w) -> p s h w", s=NS, h=6, w=12)
                for kw in range(2):
                    mm = tensor.matmul(
                        psum2[:],
                        wb_t[0:25, 48 + kw * 6 : 48 + (kw + 1) * 6],
                        h14[:, :, :, kw : kw + 11 : 2],
                        start=(kw == 0),
                        stop=(kw == 1),
                    )
                    if kw == 1:
                        mm.then_inc(psem)  # psem 2
                # fc4: 9 (hp,wp) matmuls vs the h/w-pooled tile; d-pooling and
                # /8 live in w4p; j=0 has K=7 incl. the b4 ones-row
                tensor.wait_ge(vsem, 2)
                tensor.wait_ge(dsE, 16)
                pool4 = pool_t[:].rearrange("p (s j) -> p s j", s=NS, j=9)
                for j in range(9):
                    kk = 7 if j == 0 else 6
                    mm = tensor.matmul(
                        psum4[:],
                        w4p_t[0:kk, j * 80 : (j + 1) * 80],
                        pool4[0:kk, :, j],
                        start=(j == 0),
                        stop=(j == 8),
                    )
                    if j == 8:
                        mm.then_inc(psem)  # psem 3
                # fc5
                tensor.wait_ge(asem, 3)
                tensor.wait_ge(dsF, 16)
                tensor.matmul(
                    psum5[:, 0:NS], w5t_t[0:81, 0:100], h4_t[:], start=True, stop=True
                ).then_inc(psem)  # psem 4
                tensor.matmul(
                    psum5[:, NS : 2 * NS], w5t_t[0:81, 100:200], h4_t[:], start=True, stop=True
                ).then_inc(psem)  # psem 5
                # fc6: 13 i-chunks x 2 k-halves (M=52 keeps LDWEIGHTS at its
                # 2-rows/cycle fast path; M>64 halves load throughput)
                tensor.wait_ge(asem, 4)
                tensor.wait_ge(dsG, 64)
                for i in range(13):
                    tensor.matmul(
                        psum6[:, i * NS : (i + 1) * NS],
                        w6a_t[0:100, i * 52 : (i + 1) * 52],
                        t01[0:100, 0:NS],
                        start=True,
                        stop=False,
                    )
                    mm = tensor.matmul(
                        psum6[:, i * NS : (i + 1) * NS],
                        w6b_t[0:101, i * 52 : (i + 1) * 52],
                        t01[:, NS : 2 * NS],
                        start=False,
                        stop=True,
                    )
                    if i == 12:
                        mm.then_inc(psem)  # psem 6
                # einsum
                tensor.wait_ge(asem, 5)
                h6v = h6_t[:].rearrange("p (i s) -> p s i", s=NS)
                for s in range(NS):
                    tensor.matmul(
                        psume[:, s * 168 : (s + 1) * 168],
                        h6v[:, s, :],
                        x_t[0:52, s * 168 : (s + 1) * 168],
                        start=True,
                        stop=True,
                    ).then_inc(psem)  # psem 7, 8

    _strip_entry_barrier(nc)
    return nc


def _strip_entry_barrier(nc):
    f = nc.m.functions[0]
    bbs = {bb.name: bb for bb in f.blocks}
    main = bbs["main"]
    # 1) drop the init all-engine barrier (nothing reads the const-AP tiles)
    main.instructions = [
        i
        for i in main.instructions
        if not (
            i.name.startswith("barrier_")
            or getattr(i, "opcode", "") == "Drain"
            or type(i).__name__ == "InstDrain"
        )
    ]
    # 2) hoist the input-DMA triggers into main so transfers start during the
    #    preamble, before the Block-entry rendezvous
    hoisted = {bi.ins.name for bi in getattr(nc, "_hoist_insts", [])}
    if hoisted:
        moved = []
        for bb in f.blocks:
            if bb.name == "main" or not bb.instructions:
                continue
            keep = []
            for i in bb.instructions:
                (moved if i.name in hoisted else keep).append(i)
            if len(keep) != len(bb.instructions):
                bb.instructions = keep
        # insert at the very top of main (after the entry Call): the DMA
        # triggers use only immediates + the parameter table, not the
        # preamble registers
        insts = main.instructions
        main.instructions = insts[:1] + moved + insts[1:]


def _prep_weights(w1, b1, w2, b2, w4, b4, w5, b5, w6, b6):
    f = np.float32
    w1v = np.asarray(w1, f)[0, 0]  # (6,2,2)
    w2v = np.asarray(w2, f)[0, 0, :, 0, :]  # (4,2)
    w4 = np.asarray(w4, f)
    w5 = np.asarray(w5, f)
    w6 = np.asarray(w6, f)
    b1 = np.asarray(b1, f)
    b2 = np.asarray(b2, f)
    b4 = np.asarray(b4, f)
    b5 = np.asarray(b5, f)
    b6 = np.asarray(b6, f)

    wb = np.zeros((105, 60), f)
    for kw in range(2):
        for d in range(24):
            for kd in range(6):
                wb[2 * d + kd, kw * 24 + d] = w1v[kd, 0, kw]
                wb[53 + 2 * d + kd, kw * 24 + d] = w1v[kd, 1, kw]
    wb[52, 0:24] = b1[0]  # ones-row bias, kw=0 block only
    for kd in range(4):
        for kw in range(2):
            for d in range(6):
                wb[4 * d + kd, 48 + kw * 6 + d] = w2v[kd, kw]
    wb[24, 48:54] = b2[0]  # ones-row bias, kw=0 block only

    w4r = w4.reshape(80, 3, 3, 3) / 8.0
    w4q = np.transpose(w4r, (1, 2, 3, 0)).reshape(3, 720)
    w4p = np.zeros((12, 720), f)
    w4p[0:6:2, :] = w4q
    w4p[1:6:2, :] = w4q
    w4p[6, 0:80] = b4  # ones-row bias, j=0 block only

    w5t = np.zeros((96, 200), f)
    w5t[0:80, :] = w5.T
    w5t[80, :] = b5

    w6a = np.zeros((112, 676), f)
    w6a[0:100, :] = w6[:, 0:100].T
    w6b = np.zeros((112, 676), f)
    w6b[0:100, :] = w6[:, 100:200].T
    w6b[100, :] = b6

    def _bf(a):
        return np.ascontiguousarray(a.astype(BF16))

    return dict(
        wb=_bf(wb),
        w4p=_bf(w4p),
        w5t=_bf(w5t),
        w6a1=_bf(w6a[:, 0:338]),
        w6a2=_bf(w6a[:, 338:676]),
        w6b1=_bf(w6b[:, 0:338]),
        w6b2=_bf(w6b[:, 338:676]),
    )


def kernel(x, w1, b1, w2, b2, w4, b4, w5, b5, w6, b6, _trace=False):
    global LAST_EXEC_NS, LAST_RESULT
    from concourse.bass_utils import run_bass_kernel_spmd

    if "nc" not in _BUILT:
        _BUILT["nc"] = _build_nc()
    nc = _BUILT["nc"]

    xs = np.ascontiguousarray(np.asarray(x, np.float32).reshape(10, 52, 7, 24))
    # h-shifted copy for the stacked kh=1 conv1 taps
    xsh = np.zeros_like(xs)
    xsh[:, :, 0:6, :] = xs[:, :, 1:7, :]
    xs2 = xs.reshape(10, 52, 168)
    xsh2 = xsh.reshape(10, 52, 168)
    wd = _prep_weights(w1, b1, w2, b2, w4, b4, w5, b5, w6, b6)

    in_maps = []
    for i in range(N_CORES):
        xc = np.ones((105, NS * 168), np.float32)
        xc[0:52] = np.transpose(
            np.stack([xs2[a] for a in ASSIGN[i]]), (1, 0, 2)
        ).reshape(52, NS * 168)
        xc[53:105] = np.transpose(
            np.stack([xsh2[a] for a in ASSIGN[i]]), (1, 0, 2)
        ).reshape(52, NS * 168)
        xc = np.ascontiguousarray(xc.astype(BF16))
        m = {"x": xc}
        m.update(wd)
        in_maps.append(m)

    res = run_bass_kernel_spmd(nc, in_maps, core_ids=list(range(N_CORES)), trace=_trace)
    LAST_EXEC_NS = res.exec_time_ns
    LAST_RESULT = res

    out = np.zeros((10, 2184), np.float32)
    for i in range(N_CORES):
        o = res.results[i]["out"].reshape(13, NS, 168)
        for slot, b in enumerate(ASSIGN[i]):
            out[b] = o[:, slot, :].reshape(2184)
    return out
